# revision 2
# baseline (speedup 1.0000x reference)
"""DeformConv2d Trainium2 Bass kernel, v2.

vs baseline:
  - cj/uy/ux quartered per (mp, mi) so replication DMAs start ~12us in
    (tile-granular deps no longer serialize on the whole cj tensor).
  - 9 hat-product rows per (k, mi) split across engines:
      ty0/ty1 triples -> DVE products on bf16 cj (fast 2x mode)
      ty1/ty2 triples on k%3==0 -> DVE products on fp8 cj (DMA bytes halved)
      ty2 triple otherwise       -> Pool products on fp8 cj
  - products fused per ty-triple: one [128, 3, 8, 128] tensor_mul.
  - cjr replication DMAs issue from SP/Act HWDGE queues (no Pool descgen).
  - phase-1 for quarters q2/q3 interleaved into mp0's modulation stream.

Sharding: 8 cores = (batch b in 0..3) x (row-half in 0..1), as baseline.
"""

import sys
import numpy as np
import ml_dtypes

sys.path.insert(0, "/opt/trn_rl_repo")

B, C, H, W = 4, 64, 128, 128
O = 64
NCORES = 8

_cached = {}


def _assign(k, mi, ty):
    """engine/dtype tag for the ty-triple of slot (k, mi): 'db' bf16-DVE,
    'd8' fp8-DVE, 'p8' fp8-Pool. Pattern A/B/A/C keeps adjacent slots'
    bottleneck engines different so shallow rings can pipeline them."""
    v = (k + mi) % 4
    if k == 8:
        v = 3  # no Pool work in the last tap: fast drain into the mp barrier
    if ty == 0:
        return "db"
    if ty == 1:
        return "d8" if v == 1 else "db"
    return "d8" if v == 3 else "p8"


def build_program():
    if "nc" in _cached:
        return _cached["nc"]
    import concourse.bass as bass
    import concourse.tile as tile
    import concourse.mybir as mybir
    from contextlib import ExitStack
    import bass_rust as _br

    dt = mybir.dt
    AF = mybir.ActivationFunctionType
    ALU = mybir.AluOpType

    nc = bass.Bass()

    xe_d = nc.dram_tensor("xe", [128, 72, 136], dt.bfloat16, kind="ExternalInput")
    wop_d = nc.dram_tensor("wop", [128, 3, 18], dt.bfloat16, kind="ExternalInput")
    woff_d = nc.dram_tensor("woff", [64, 9, 18], dt.bfloat16, kind="ExternalInput")
    ob_d = nc.dram_tensor("obc", [18, 1], dt.float32, kind="ExternalInput")
    sela_d = nc.dram_tensor("sela", [18, 81], dt.bfloat16, kind="ExternalInput")
    selb_d = nc.dram_tensor("selb", [18, 81], dt.bfloat16, kind="ExternalInput")
    nty_d = nc.dram_tensor("nty", [81, 1], dt.float32, kind="ExternalInput")
    ntx_d = nc.dram_tensor("ntx", [81, 1], dt.float32, kind="ExternalInput")
    wk_d = nc.dram_tensor("wk", [64, 9, 64], dt.bfloat16, kind="ExternalInput")
    ident_d = nc.dram_tensor("ident", [128, 128], dt.bfloat16, kind="ExternalInput")
    bcol_d = nc.dram_tensor("bcol", [128, 1], dt.float32, kind="ExternalInput")
    out_d = nc.dram_tensor("out", [128, 4096], dt.float32, kind="ExternalOutput")

    with tile.TileContext(nc) as tc, ExitStack() as ctx:
        const_pool = ctx.enter_context(tc.tile_pool(name="consts", bufs=1))
        xe = const_pool.tile([128, 72, 136], dt.bfloat16)
        nc.sync.dma_start(xe[:], xe_d[:])
        wop = const_pool.tile([128, 3, 18], dt.bfloat16)
        nc.sync.dma_start(wop[:], wop_d[:])
        woff = const_pool.tile([64, 9, 18], dt.bfloat16)
        nc.sync.dma_start(woff[:], woff_d[:])
        obc = const_pool.tile([18, 1], dt.float32)
        nc.sync.dma_start(obc[:], ob_d[:])
        sela = const_pool.tile([18, 81], dt.bfloat16)
        nc.sync.dma_start(sela[:], sela_d[:])
        selb = const_pool.tile([18, 81], dt.bfloat16)
        nc.sync.dma_start(selb[:], selb_d[:])
        nty = const_pool.tile([81, 1], dt.float32)
        nc.sync.dma_start(nty[:], nty_d[:])
        ntx = const_pool.tile([81, 1], dt.float32)
        nc.sync.dma_start(ntx[:], ntx_d[:])
        wk = const_pool.tile([64, 9, 64], dt.bfloat16)
        nc.sync.dma_start(wk[:], wk_d[:])
        ident = const_pool.tile([128, 128], dt.bfloat16)
        nc.sync.dma_start(ident[:], ident_d[:])
        bcol = const_pool.tile([128, 1], dt.float32)
        nc.sync.dma_start(bcol[:], bcol_d[:])

        # per-quarter cj tensors. cjq [81, 2048] = [ph0 1024 | ph1 1024] is
        # the hat-product staging layout; a SWDGE reshape turns it into
        # cjT2 [54 = (k,ty)x2+ph, 3072 = (tx,f)] whose free-contiguous
        # triples make the o-replication DMAs legal 3-dim HWDGE copies.
        cjq_pool = ctx.enter_context(tc.tile_pool(name="cjq", bufs=2))
        cj_pool = ctx.enter_context(tc.tile_pool(name="cj", bufs=1))
        cjT = [cj_pool.tile([54, 3, 1024], dt.bfloat16, name=f"cjT{g}")
               for g in range(4)]
        cjT8 = [cj_pool.tile([54, 3, 1024], dt.float8e4, name=f"cjT8{g}")
                for g in range(4)]

        op_pool = ctx.enter_context(tc.tile_pool(name="p1psum", bufs=2,
                                                 space="PSUM"))
        sb1_pool = ctx.enter_context(tc.tile_pool(name="p1sb", bufs=2))

        def phase1_quarter(g):
            # chunks of this quarter: ph0: 2g, 2g+1 ; ph1: 8+2g, 8+2g+1
            offs_sb = sb1_pool.tile([18, 4, 512], dt.bfloat16, tag="offs",
                                    name=f"offs{g}")
            uyq = sb1_pool.tile([81, 2048], dt.bfloat16, tag="uy",
                                name=f"uy{g}")
            uxq = sb1_pool.tile([81, 2048], dt.bfloat16, tag="ux",
                                name=f"ux{g}")
            chs = [2 * g, 2 * g + 1, 8 + 2 * g, 8 + 2 * g + 1]
            for ci, ch in enumerate(chs):
                po = op_pool.tile([81, 512], dt.float32, tag="p1", name="po")
                for ky in range(3):
                    ay = ky - 1
                    rhs = xe[:, 4 + 4 * ch + ay : 4 + 4 * ch + ay + 4,
                             3 : 3 + 128]
                    nc.tensor.matmul(po[0:18, :], wop[:, ky, :], rhs,
                                     start=(ky == 0), stop=False)
                for ky in range(3):
                    ay = ky - 1
                    rhs = xe[0:64, 4 + 4 * ch + ay : 4 + 4 * ch + ay + 4,
                             5 : 5 + 128]
                    nc.tensor.matmul(po[0:18, :], woff[:, 3 * ky + 2, :], rhs,
                                     start=False, stop=(ky == 2))
                nc.scalar.activation(offs_sb[:, ci, :], po[0:18, :],
                                     AF.Identity, bias=obc[:], scale=1.0)
                pa = op_pool.tile([81, 512], dt.float32, tag="p1", name="pa")
                nc.tensor.matmul(pa[:], sela[:], offs_sb[:, ci, :],
                                 start=True, stop=True)
                nc.scalar.activation(uyq[:, 512 * ci : 512 * (ci + 1)], pa[:],
                                     AF.Abs, bias=nty[:], scale=1.0)
                pb = op_pool.tile([81, 512], dt.float32, tag="p1", name="pb")
                nc.tensor.matmul(pb[:], selb[:], offs_sb[:, ci, :],
                                 start=True, stop=True)
                nc.scalar.activation(uxq[:, 512 * ci : 512 * (ci + 1)], pb[:],
                                     AF.Abs, bias=ntx[:], scale=1.0)
            nc.vector.tensor_scalar(uyq[:], uyq[:], 1.0, 0.0,
                                    ALU.subtract, ALU.min)
            nc.vector.tensor_scalar(uxq[:], uxq[:], 1.0, 0.0,
                                    ALU.subtract, ALU.min)
            cjq = cjq_pool.tile([81, 2048], dt.bfloat16, tag="cjq",
                                name=f"cjq{g}")
            nc.vector.tensor_mul(cjq[:], uyq[:], uxq[:])
            # reshape [t=9k+3ty+tx, (ph,f)] -> [27*ph + 3k+ty, tx, f]:
            # one DMA per (ph, tx): dim0-only partition strides (HWDGE ok)
            for ph in range(2):
                for j in range(3):
                    src_ap = cjq[j : min(j + 79, 81),
                                 1024 * ph : 1024 * (ph + 1)].copy()
                    sp = src_ap.ap[0][0]
                    src_ap.ap = _br.VecI64Pair([[3 * sp, 27], [1, 1024]])
                    nc.sync.dma_start(
                        cjT[g][27 * ph : 27 * ph + 27, j, :], src_ap)
            nc.scalar.activation(cjT8[g][:], cjT[g][:], AF.Copy, scale=1.0)

        zp_pool = ctx.enter_context(tc.tile_pool(name="zpsum", bufs=2,
                                                 space="PSUM"))
        acc_pool = ctx.enter_context(tc.tile_pool(name="acc", bufs=1,
                                                  space="PSUM"))
        zblk_pool = ctx.enter_context(tc.tile_pool(name="zblk", bufs=3))
        cjr_pool = ctx.enter_context(tc.tile_pool(name="cjr", bufs=4))
        cjr8p_pool = ctx.enter_context(tc.tile_pool(name="cjr8p", bufs=6))
        prod_pool = ctx.enter_context(tc.tile_pool(name="prod", bufs=3))
        prods_pool = ctx.enter_context(tc.tile_pool(name="prods", bufs=6))
        outsb_pool = ctx.enter_context(tc.tile_pool(name="outsb", bufs=1))

        def emit_zblk(mp, k):
            zblk = zblk_pool.tile([128, 20, 132], dt.bfloat16, name="zblk")
            for rc in range(5):
                zp = zp_pool.tile([128, 512], dt.float32, tag="zp", name="zp")
                for ph in range(2):
                    xrow = ph * 32 + 16 * mp + 4 * rc + 2
                    rhs = xe[0:64, xrow : xrow + 4, 2 : 2 + 128]
                    nc.tensor.matmul(zp[64 * ph : 64 * ph + 64, :], wk[:, k, :],
                                     rhs, start=True, stop=True)
                nc.scalar.activation(
                    zblk[:, 4 * rc : 4 * rc + 4, 0:128],
                    zp[:].rearrange("p (a b) -> p a b", a=4), AF.Copy,
                    scale=1.0)
            zps = zp_pool.tile([128, 20, 4], dt.float32, tag="zp", name="zps")
            for ph in range(2):
                xrow = ph * 32 + 16 * mp + 2
                rhs = xe[0:64, xrow : xrow + 20, 130 : 134]
                nc.tensor.matmul(zps[64 * ph : 64 * ph + 64, :, :], wk[:, k, :],
                                 rhs, start=True, stop=True)
            nc.scalar.activation(zblk[:, :, 128:132], zps[:], AF.Copy,
                                 scale=1.0)
            return zblk

        def rep_dma(queue_dma, dst, srcT, k, ty):
            """replicate the (k, ty) triple (rows 3k+ty and 27+3k+ty of
            srcT, free [tx, f] = 3072) into dst [128, 3, 1024]: partition
            p = ph*64+o reads srcT row 27*ph + 3k+ty. The 28-row slice
            over-claims the region (safe: conservative hazards only)."""
            r0 = 3 * k + ty
            src_ap = srcT[r0 : r0 + 28, :, :].copy()
            pitch = src_ap.ap[0][0]
            src_ap.ap = _br.VecI64Pair([[27 * pitch, 2], [0, 64], [1, 3072]])
            queue_dma(dst[:], src_ap)

        def emit_prods(mp, k, mi, zblk, srcs):
            """srcs: {ty: (tile, row_ofs, engine_tag)}. Emits the DVE/Pool
            products; returns the per-triple prod list for emit_accums."""
            ky, kx = k // 3, k % 3
            out = []
            for ty in range(3):
                src, ofs, tag = srcs[ty]
                rb = 8 * mi + ky + ty
                pool_op = tag == "p8"
                # Pool runs singles (a 6us fused triple would stall the PE
                # accum stream); DVE runs the fused triple.
                tls = [(0, 3)] if not pool_op else [(0, 1), (1, 2), (2, 3)]
                for lo, hi in tls:
                    nt = hi - lo
                    if pool_op:
                        prod = prods_pool.tile([128, 1, 8, 128], dt.bfloat16,
                                               name="prodS")
                    else:
                        prod = prod_pool.tile([128, 3, 8, 128], dt.bfloat16,
                                              name="prod")
                    # slice claims the exact read region (cols kx+lo ..
                    # +127+nt) so hazards cover the zblk edge-column writes
                    z_ap = zblk[:, rb : rb + 8,
                                kx + lo : kx + lo + 127 + nt].copy()
                    zpitch = z_ap.ap[0][0]
                    z_ap.ap = _br.VecI64Pair(
                        [[zpitch, 128], [1, nt], [132, 8], [1, 128]])
                    c_ap = src[:, ofs + lo : ofs + hi, :]
                    eng = nc.gpsimd if pool_op else nc.vector
                    eng.tensor_mul(prod[:, 0:nt, :, :], z_ap, c_ap)
                    out.append((prod, ty, lo, nt))
            return out

        def emit_accums(k, prods, acc):
            first_k = (k == 0)
            for prod, ty, lo, nt in prods:
                for tl in range(nt):
                    pf = prod[:, tl, :, :].rearrange("p a b -> p (a b)")
                    for nchunk in range(2):
                        nc.tensor.matmul(
                            acc[:, 512 * nchunk : 512 * (nchunk + 1)],
                            ident[:],
                            pf[:, 512 * nchunk : 512 * (nchunk + 1)],
                            start=(first_k and ty == 0 and tl + lo == 0),
                            stop=(k == 8 and ty == 2 and tl + lo == 2),
                            skip_group_check=True)

        # ---------------- schedule ----------------------------------------
        phase1_quarter(0)
        phase1_quarter(1)

        for mp in range(2):
            acc0 = acc_pool.tile([128, 1024], dt.float32, tag="acc0")
            acc1 = acc_pool.tile([128, 1024], dt.float32, tag="acc1")
            accs = [acc0, acc1]
            zblks = {0: emit_zblk(mp, 0), 1: emit_zblk(mp, 1)}
            pending = None  # (k, prods, acc) accums lag one slot behind
            for k in range(9):
                # hoist zblk two taps ahead: its PE matmuls land before
                # accums(k) in the PE queue, its Act evacs overlap them, so
                # the k+2 products never wait on zblk latency
                if k + 2 <= 8:
                    zblks[k + 2] = emit_zblk(mp, k + 2)
                if mp == 0 and k == 2:
                    phase1_quarter(2)
                if mp == 0 and k == 5:
                    phase1_quarter(3)
                zblk = zblks.pop(k)
                for mi in range(2):
                    g = 2 * mp + mi
                    srcs = {}
                    for ty in range(3):
                        tag = _assign(k, mi, ty)
                        if tag == "db":
                            cjr = cjr_pool.tile([128, 3, 1024], dt.bfloat16,
                                                tag="cjrB", name="cjrB")
                            rep_dma(nc.sync.dma_start, cjr, cjT[g], k, ty)
                        elif tag == "d8":
                            cjr = cjr_pool.tile([128, 3, 1024], dt.float8e4,
                                                tag="cjr8d", name="cjr8d")
                            rep_dma(nc.sync.dma_start, cjr, cjT8[g], k, ty)
                        else:
                            cjr = cjr8p_pool.tile([128, 3, 1024], dt.float8e4,
                                                  tag="cjr8p", name="cjr8p")
                            rep_dma(nc.sync.dma_start, cjr, cjT8[g], k, ty)
                        srcs[ty] = (cjr, 0, tag)
                    prods = emit_prods(mp, k, mi, zblk, srcs)
                    if pending is not None:
                        emit_accums(*pending)
                    pending = (k, prods, accs[mi])
            emit_accums(*pending)

            for mi in range(2):
                m = 2 * mp + mi
                osb = outsb_pool.tile([128, 1024], dt.float32)
                nc.scalar.activation(osb[:], accs[mi][:], AF.Identity,
                                     bias=bcol[:], scale=1.0)
                nc.sync.dma_start(out_d[:, 1024 * m : 1024 * (m + 1)], osb[:])

    _patch_multiwait(nc)
    _cached["nc"] = nc
    return nc


def _patch_multiwait(nc):
    """walrus here accepts one sync-wait per instruction; split extras onto
    injected same-engine Drain carriers (waiting earlier is always safe)."""
    import json
    import types

    orig = nc.to_json_bytes

    def patched(self):
        bir = json.loads(orig())
        uid = [0]
        for fn in bir["functions"]:
            for blk in fn["blocks"]:
                out = []
                for ins in blk["instructions"]:
                    si = ins.get("sync_info")
                    ow = (si or {}).get("on_wait") or []
                    if len(ow) > 1:
                        for w in ow[:-1]:
                            uid[0] += 1
                            out.append({
                                "debug": ins.get("debug", 0),
                                "engine": ins["engine"],
                                "ins": [], "outs": [],
                                "name": f"WSPL-{uid[0]}",
                                "opcode": "Drain",
                                "sync_info": {"on_update": [],
                                              "on_wait": [w]},
                            })
                        si["on_wait"] = [ow[-1]]
                    out.append(ins)
                blk["instructions"] = out
        return json.dumps(bir).encode()

    nc.to_json_bytes = types.MethodType(patched, nc)


def _host_inputs(x, offset_w, offset_b, weight, bias):
    bf16 = ml_dtypes.bfloat16
    woff = np.ascontiguousarray(
        offset_w.reshape(18, 64, 9).transpose(1, 2, 0)
    ).astype(bf16)
    obc = offset_b.reshape(18, 1).astype(np.float32)
    sela = np.zeros((18, 81), np.float32)
    selb = np.zeros((18, 81), np.float32)
    nty = np.zeros((81, 1), np.float32)
    ntx = np.zeros((81, 1), np.float32)
    for k in range(9):
        for i, tyv in enumerate((-1, 0, 1)):
            for j, txv in enumerate((-1, 0, 1)):
                t = 9 * k + 3 * i + j
                sela[2 * k, t] = 1.0
                selb[2 * k + 1, t] = 1.0
                nty[t, 0] = -float(tyv)
                ntx[t, 0] = -float(txv)
    sela = sela.astype(bf16)
    selb = selb.astype(bf16)
    wk = np.ascontiguousarray(
        weight.reshape(64, 64, 9).transpose(1, 2, 0)
    ).astype(bf16)
    wop = np.zeros((128, 3, 18), np.float32)
    wop[0:64] = woff.astype(np.float32)[:, 0::3, :]
    wop[64:128] = woff.astype(np.float32)[:, 1::3, :]
    wop = wop.astype(bf16)
    ident = np.eye(128, dtype=np.float32).astype(bf16)
    bcol = np.tile(bias, 2).reshape(128, 1).astype(np.float32)

    in_maps = []
    for core in range(NCORES):
        bb, half = core // 2, core % 2
        r0 = 64 * half
        xe = np.zeros((128, 72, 136), np.float32)
        rlo, rhi = r0 - 4, r0 + 68
        slo, shi = max(rlo, 0), min(rhi, H)
        xe[0:64, slo - rlo : shi - rlo, 4 : 4 + W] = x[bb, :, slo:shi, :]
        xe[64:128, :, 0:135] = xe[0:64, :, 1:136]
        in_maps.append(dict(
            xe=xe.astype(bf16), woff=woff, wop=wop, obc=obc, sela=sela,
            selb=selb, nty=nty, ntx=ntx, wk=wk, ident=ident, bcol=bcol,
        ))
    return in_maps


def kernel(x, offset_w, offset_b, weight, bias):
    x = np.asarray(x, np.float32)
    offset_w = np.asarray(offset_w, np.float32)
    offset_b = np.asarray(offset_b, np.float32)
    weight = np.asarray(weight, np.float32)
    bias = np.asarray(bias, np.float32)

    from concourse.bass_utils import run_bass_kernel_spmd

    nc = build_program()
    in_maps = _host_inputs(x, offset_w, offset_b, weight, bias)
    res = run_bass_kernel_spmd(nc, in_maps, core_ids=list(range(NCORES)))
    _cached["exec_time_ns"] = res.exec_time_ns

    out = np.zeros((B, O, H, W), np.float32)
    for core in range(NCORES):
        raw = res.results[core]["out"]
        bb, half = core // 2, core % 2
        r0 = 64 * half
        v = raw.reshape(2, 64, 4, 8, 128)
        v = v.transpose(1, 0, 2, 3, 4).reshape(64, 64, 128)
        out[bb, :, r0 : r0 + 64, :] = v
    return out


if __name__ == "__main__":
    xs = {
        "x": np.random.randn(B, C, H, W).astype(np.float32),
        "offset_w": (np.random.randn(18, 64, 3, 3) * 0.01).astype(np.float32),
        "offset_b": (np.random.randn(18) * 0.01).astype(np.float32),
        "weight": (np.random.randn(64, 64, 3, 3) / np.sqrt(576)).astype(np.float32),
        "bias": (np.random.randn(64) * 0.01).astype(np.float32),
    }
    r = kernel(**xs)
    print(r.shape, np.abs(r).max())


# revision 3
# speedup vs baseline: 1.0245x; 1.0245x over previous
"""DeformConv2d Trainium2 Bass kernel, v2.

vs baseline:
  - cj/uy/ux quartered per (mp, mi) so replication DMAs start ~12us in
    (tile-granular deps no longer serialize on the whole cj tensor).
  - 9 hat-product rows per (k, mi) split across engines:
      ty0/ty1 triples -> DVE products on bf16 cj (fast 2x mode)
      ty1/ty2 triples on k%3==0 -> DVE products on fp8 cj (DMA bytes halved)
      ty2 triple otherwise       -> Pool products on fp8 cj
  - products fused per ty-triple: one [128, 3, 8, 128] tensor_mul.
  - cjr replication DMAs issue from SP/Act HWDGE queues (no Pool descgen).
  - phase-1 for quarters q2/q3 interleaved into mp0's modulation stream.

Sharding: 8 cores = (batch b in 0..3) x (row-half in 0..1), as baseline.
"""

import sys
import numpy as np
import ml_dtypes

sys.path.insert(0, "/opt/trn_rl_repo")

B, C, H, W = 4, 64, 128, 128
O = 64
NCORES = 8

_cached = {}


def _assign(k, mi, ty):
    """engine/dtype tag for the ty-triple of slot (k, mi): 'db' bf16-DVE,
    'd8' fp8-DVE, 'p8' fp8-Pool. Pattern A/B/A/C keeps adjacent slots'
    bottleneck engines different so shallow rings can pipeline them."""
    v = (k + mi) % 4
    if k == 8:
        v = 3  # no Pool work in the last tap: fast drain into the mp barrier
    if ty == 0:
        return "db"
    if ty == 1:
        return "d8" if v == 1 else "db"
    return "d8" if v == 3 else "p8"


def build_program():
    if "nc" in _cached:
        return _cached["nc"]
    import concourse.bass as bass
    import concourse.tile as tile
    import concourse.mybir as mybir
    from contextlib import ExitStack
    import bass_rust as _br

    dt = mybir.dt
    AF = mybir.ActivationFunctionType
    ALU = mybir.AluOpType

    nc = bass.Bass()

    xe_d = nc.dram_tensor("xe", [128, 72, 136], dt.bfloat16, kind="ExternalInput")
    wop_d = nc.dram_tensor("wop", [128, 3, 18], dt.bfloat16, kind="ExternalInput")
    woff_d = nc.dram_tensor("woff", [64, 9, 18], dt.bfloat16, kind="ExternalInput")
    ob_d = nc.dram_tensor("obc", [18, 1], dt.float32, kind="ExternalInput")
    sela_d = nc.dram_tensor("sela", [18, 81], dt.bfloat16, kind="ExternalInput")
    selb_d = nc.dram_tensor("selb", [18, 81], dt.bfloat16, kind="ExternalInput")
    nty_d = nc.dram_tensor("nty", [81, 1], dt.float32, kind="ExternalInput")
    ntx_d = nc.dram_tensor("ntx", [81, 1], dt.float32, kind="ExternalInput")
    wk_d = nc.dram_tensor("wk", [64, 9, 64], dt.bfloat16, kind="ExternalInput")
    ident_d = nc.dram_tensor("ident", [128, 128], dt.bfloat16, kind="ExternalInput")
    bcol_d = nc.dram_tensor("bcol", [128, 1], dt.float32, kind="ExternalInput")
    out_d = nc.dram_tensor("out", [128, 4096], dt.float32, kind="ExternalOutput")

    with tile.TileContext(nc) as tc, ExitStack() as ctx:
        const_pool = ctx.enter_context(tc.tile_pool(name="consts", bufs=1))
        xe = const_pool.tile([128, 72, 136], dt.bfloat16)
        nc.sync.dma_start(xe[:], xe_d[:])
        wop = const_pool.tile([128, 3, 18], dt.bfloat16)
        nc.sync.dma_start(wop[:], wop_d[:])
        woff = const_pool.tile([64, 9, 18], dt.bfloat16)
        nc.sync.dma_start(woff[:], woff_d[:])
        obc = const_pool.tile([18, 1], dt.float32)
        nc.sync.dma_start(obc[:], ob_d[:])
        sela = const_pool.tile([18, 81], dt.bfloat16)
        nc.sync.dma_start(sela[:], sela_d[:])
        selb = const_pool.tile([18, 81], dt.bfloat16)
        nc.sync.dma_start(selb[:], selb_d[:])
        nty = const_pool.tile([81, 1], dt.float32)
        nc.sync.dma_start(nty[:], nty_d[:])
        ntx = const_pool.tile([81, 1], dt.float32)
        nc.sync.dma_start(ntx[:], ntx_d[:])
        wk = const_pool.tile([64, 9, 64], dt.bfloat16)
        nc.sync.dma_start(wk[:], wk_d[:])
        ident = const_pool.tile([128, 128], dt.bfloat16)
        nc.sync.dma_start(ident[:], ident_d[:])
        bcol = const_pool.tile([128, 1], dt.float32)
        nc.sync.dma_start(bcol[:], bcol_d[:])

        # per-quarter cj tensors. cjq [81, 2048] = [ph0 1024 | ph1 1024] is
        # the hat-product staging layout; a SWDGE reshape turns it into
        # cjT2 [54 = (k,ty)x2+ph, 3072 = (tx,f)] whose free-contiguous
        # triples make the o-replication DMAs legal 3-dim HWDGE copies.
        cjq_pool = ctx.enter_context(tc.tile_pool(name="cjq", bufs=2))
        cj_pool = ctx.enter_context(tc.tile_pool(name="cj", bufs=1))
        cjT = [cj_pool.tile([54, 3, 1024], dt.bfloat16, name=f"cjT{g}")
               for g in range(4)]
        cjT8 = [cj_pool.tile([54, 3, 1024], dt.float8e4, name=f"cjT8{g}")
                for g in range(4)]

        op_pool = ctx.enter_context(tc.tile_pool(name="p1psum", bufs=2,
                                                 space="PSUM"))
        sb1_pool = ctx.enter_context(tc.tile_pool(name="p1sb", bufs=2))

        def phase1_quarter(g):
            # chunks of this quarter: ph0: 2g, 2g+1 ; ph1: 8+2g, 8+2g+1
            offs_sb = sb1_pool.tile([18, 4, 512], dt.bfloat16, tag="offs",
                                    name=f"offs{g}")
            uyq = sb1_pool.tile([81, 2048], dt.bfloat16, tag="uy",
                                name=f"uy{g}")
            uxq = sb1_pool.tile([81, 2048], dt.bfloat16, tag="ux",
                                name=f"ux{g}")
            chs = [2 * g, 2 * g + 1, 8 + 2 * g, 8 + 2 * g + 1]
            for ci, ch in enumerate(chs):
                po = op_pool.tile([81, 512], dt.float32, tag="p1", name="po")
                for ky in range(3):
                    ay = ky - 1
                    rhs = xe[:, 4 + 4 * ch + ay : 4 + 4 * ch + ay + 4,
                             3 : 3 + 128]
                    nc.tensor.matmul(po[0:18, :], wop[:, ky, :], rhs,
                                     start=(ky == 0), stop=False)
                for ky in range(3):
                    ay = ky - 1
                    rhs = xe[0:64, 4 + 4 * ch + ay : 4 + 4 * ch + ay + 4,
                             5 : 5 + 128]
                    nc.tensor.matmul(po[0:18, :], woff[:, 3 * ky + 2, :], rhs,
                                     start=False, stop=(ky == 2))
                nc.scalar.activation(offs_sb[:, ci, :], po[0:18, :],
                                     AF.Identity, bias=obc[:], scale=1.0)
                pa = op_pool.tile([81, 512], dt.float32, tag="p1", name="pa")
                nc.tensor.matmul(pa[:], sela[:], offs_sb[:, ci, :],
                                 start=True, stop=True)
                nc.scalar.activation(uyq[:, 512 * ci : 512 * (ci + 1)], pa[:],
                                     AF.Abs, bias=nty[:], scale=1.0)
                pb = op_pool.tile([81, 512], dt.float32, tag="p1", name="pb")
                nc.tensor.matmul(pb[:], selb[:], offs_sb[:, ci, :],
                                 start=True, stop=True)
                nc.scalar.activation(uxq[:, 512 * ci : 512 * (ci + 1)], pb[:],
                                     AF.Abs, bias=ntx[:], scale=1.0)
            nc.vector.tensor_scalar(uyq[:], uyq[:], 1.0, 0.0,
                                    ALU.subtract, ALU.min)
            nc.vector.tensor_scalar(uxq[:], uxq[:], 1.0, 0.0,
                                    ALU.subtract, ALU.min)
            cjq = cjq_pool.tile([81, 2048], dt.bfloat16, tag="cjq",
                                name=f"cjq{g}")
            nc.vector.tensor_mul(cjq[:], uyq[:], uxq[:])
            # reshape [t=9k+3ty+tx, (ph,f)] -> [27*ph + 3k+ty, tx, f]:
            # one DMA per (ph, tx): dim0-only partition strides (HWDGE ok)
            for ph in range(2):
                for j in range(3):
                    src_ap = cjq[j : min(j + 79, 81),
                                 1024 * ph : 1024 * (ph + 1)].copy()
                    sp = src_ap.ap[0][0]
                    src_ap.ap = _br.VecI64Pair([[3 * sp, 27], [1, 1024]])
                    nc.sync.dma_start(
                        cjT[g][27 * ph : 27 * ph + 27, j, :], src_ap)
            nc.scalar.activation(cjT8[g][:], cjT[g][:], AF.Copy, scale=1.0)

        zp_pool = ctx.enter_context(tc.tile_pool(name="zpsum", bufs=2,
                                                 space="PSUM"))
        acc_pool = ctx.enter_context(tc.tile_pool(name="acc", bufs=1,
                                                  space="PSUM"))
        zblk_pool = ctx.enter_context(tc.tile_pool(name="zblk", bufs=3))
        cjr_pool = ctx.enter_context(tc.tile_pool(name="cjr", bufs=4))
        cjr8p_pool = ctx.enter_context(tc.tile_pool(name="cjr8p", bufs=6))
        prod_pool = ctx.enter_context(tc.tile_pool(name="prod", bufs=3))
        prods_pool = ctx.enter_context(tc.tile_pool(name="prods", bufs=6))
        outsb_pool = ctx.enter_context(tc.tile_pool(name="outsb", bufs=1))

        def emit_zblk(mp, k):
            zblk = zblk_pool.tile([128, 20, 132], dt.bfloat16, name="zblk")
            for rc in range(5):
                zp = zp_pool.tile([128, 512], dt.float32, tag="zp", name="zp")
                for ph in range(2):
                    xrow = ph * 32 + 16 * mp + 4 * rc + 2
                    rhs = xe[0:64, xrow : xrow + 4, 2 : 2 + 128]
                    nc.tensor.matmul(zp[64 * ph : 64 * ph + 64, :], wk[:, k, :],
                                     rhs, start=True, stop=True)
                nc.scalar.activation(
                    zblk[:, 4 * rc : 4 * rc + 4, 0:128],
                    zp[:].rearrange("p (a b) -> p a b", a=4), AF.Copy,
                    scale=1.0)
            zps = zp_pool.tile([128, 20, 4], dt.float32, tag="zp", name="zps")
            for ph in range(2):
                xrow = ph * 32 + 16 * mp + 2
                rhs = xe[0:64, xrow : xrow + 20, 130 : 134]
                nc.tensor.matmul(zps[64 * ph : 64 * ph + 64, :, :], wk[:, k, :],
                                 rhs, start=True, stop=True)
            nc.scalar.activation(zblk[:, :, 128:132], zps[:], AF.Copy,
                                 scale=1.0)
            return zblk

        def rep_dma(queue_dma, dst, srcT, k, ty):
            """replicate the (k, ty) triple (rows 3k+ty and 27+3k+ty of
            srcT, free [tx, f] = 3072) into dst [128, 3, 1024]: partition
            p = ph*64+o reads srcT row 27*ph + 3k+ty. The 28-row slice
            over-claims the region (safe: conservative hazards only)."""
            r0 = 3 * k + ty
            src_ap = srcT[r0 : r0 + 28, :, :].copy()
            pitch = src_ap.ap[0][0]
            src_ap.ap = _br.VecI64Pair([[27 * pitch, 2], [0, 64], [1, 3072]])
            queue_dma(dst[:], src_ap)

        def emit_prods(mp, k, mi, zblk, srcs):
            """srcs: {ty: (tile, row_ofs, engine_tag)}. Emits the DVE/Pool
            products; returns the per-triple prod list for emit_accums."""
            ky, kx = k // 3, k % 3
            out = []
            # Pool triples first: the slow Pool engine gets a slot of lead
            # time; DVE fills in behind it
            order = sorted(range(3), key=lambda t: srcs[t][2] != "p8")
            for ty in order:
                src, ofs, tag = srcs[ty]
                rb = 8 * mi + ky + ty
                pool_op = tag == "p8"
                # Pool runs singles (a 6us fused triple would stall the PE
                # accum stream); DVE runs the fused triple.
                tls = [(0, 3)] if not pool_op else [(0, 1), (1, 2), (2, 3)]
                for lo, hi in tls:
                    nt = hi - lo
                    if pool_op:
                        prod = prods_pool.tile([128, 1, 8, 128], dt.bfloat16,
                                               name="prodS")
                    else:
                        prod = prod_pool.tile([128, 3, 8, 128], dt.bfloat16,
                                              name="prod")
                    # slice claims the exact read region (cols kx+lo ..
                    # +127+nt) so hazards cover the zblk edge-column writes
                    z_ap = zblk[:, rb : rb + 8,
                                kx + lo : kx + lo + 127 + nt].copy()
                    zpitch = z_ap.ap[0][0]
                    z_ap.ap = _br.VecI64Pair(
                        [[zpitch, 128], [1, nt], [132, 8], [1, 128]])
                    c_ap = src[:, ofs + lo : ofs + hi, :]
                    eng = nc.gpsimd if pool_op else nc.vector
                    eng.tensor_mul(prod[:, 0:nt, :, :], z_ap, c_ap)
                    out.append((prod, ty, lo, nt))
            return out

        def emit_accums(k, prods, acc):
            nmm = sum(nt for _, _, _, nt in prods)
            i = 0
            for prod, ty, lo, nt in prods:
                for tl in range(nt):
                    i += 1
                    pf = prod[:, tl, :, :].rearrange("p a b -> p (a b)")
                    for nchunk in range(2):
                        nc.tensor.matmul(
                            acc[:, 512 * nchunk : 512 * (nchunk + 1)],
                            ident[:],
                            pf[:, 512 * nchunk : 512 * (nchunk + 1)],
                            start=(k == 0 and i == 1),
                            stop=(k == 8 and i == nmm),
                            skip_group_check=True)

        # ---------------- schedule ----------------------------------------
        phase1_quarter(0)
        phase1_quarter(1)

        for mp in range(2):
            acc0 = acc_pool.tile([128, 1024], dt.float32, tag="acc0")
            acc1 = acc_pool.tile([128, 1024], dt.float32, tag="acc1")
            accs = [acc0, acc1]
            zblks = {0: emit_zblk(mp, 0), 1: emit_zblk(mp, 1)}
            pending = None  # (k, prods, acc) accums lag one slot behind
            for k in range(9):
                # hoist zblk two taps ahead: its PE matmuls land before
                # accums(k) in the PE queue, its Act evacs overlap them, so
                # the k+2 products never wait on zblk latency
                if k + 2 <= 8:
                    zblks[k + 2] = emit_zblk(mp, k + 2)
                if mp == 0 and k == 2:
                    phase1_quarter(2)
                if mp == 0 and k == 5:
                    phase1_quarter(3)
                zblk = zblks.pop(k)
                for mi in range(2):
                    g = 2 * mp + mi
                    srcs = {}
                    tyo = sorted(range(3),
                                 key=lambda t: _assign(k, mi, t) != "p8")
                    for ty in tyo:
                        tag = _assign(k, mi, ty)
                        if tag == "db":
                            cjr = cjr_pool.tile([128, 3, 1024], dt.bfloat16,
                                                tag="cjrB", name="cjrB")
                            rep_dma(nc.sync.dma_start, cjr, cjT[g], k, ty)
                        elif tag == "d8":
                            cjr = cjr_pool.tile([128, 3, 1024], dt.float8e4,
                                                tag="cjr8d", name="cjr8d")
                            rep_dma(nc.sync.dma_start, cjr, cjT8[g], k, ty)
                        else:
                            cjr = cjr8p_pool.tile([128, 3, 1024], dt.float8e4,
                                                  tag="cjr8p", name="cjr8p")
                            rep_dma(nc.sync.dma_start, cjr, cjT8[g], k, ty)
                        srcs[ty] = (cjr, 0, tag)
                    prods = emit_prods(mp, k, mi, zblk, srcs)
                    if pending is not None:
                        emit_accums(*pending)
                    pending = (k, prods, accs[mi])
            emit_accums(*pending)

            for mi in range(2):
                m = 2 * mp + mi
                osb = outsb_pool.tile([128, 1024], dt.float32)
                nc.scalar.activation(osb[:], accs[mi][:], AF.Identity,
                                     bias=bcol[:], scale=1.0)
                nc.sync.dma_start(out_d[:, 1024 * m : 1024 * (m + 1)], osb[:])

    _patch_multiwait(nc)
    _cached["nc"] = nc
    return nc


def _patch_multiwait(nc):
    """walrus here accepts one sync-wait per instruction; split extras onto
    injected same-engine Drain carriers (waiting earlier is always safe)."""
    import json
    import types

    orig = nc.to_json_bytes

    def patched(self):
        bir = json.loads(orig())
        uid = [0]
        for fn in bir["functions"]:
            for blk in fn["blocks"]:
                out = []
                for ins in blk["instructions"]:
                    si = ins.get("sync_info")
                    ow = (si or {}).get("on_wait") or []
                    if len(ow) > 1:
                        for w in ow[:-1]:
                            uid[0] += 1
                            out.append({
                                "debug": ins.get("debug", 0),
                                "engine": ins["engine"],
                                "ins": [], "outs": [],
                                "name": f"WSPL-{uid[0]}",
                                "opcode": "Drain",
                                "sync_info": {"on_update": [],
                                              "on_wait": [w]},
                            })
                        si["on_wait"] = [ow[-1]]
                    out.append(ins)
                blk["instructions"] = out
        return json.dumps(bir).encode()

    nc.to_json_bytes = types.MethodType(patched, nc)


def _host_inputs(x, offset_w, offset_b, weight, bias):
    bf16 = ml_dtypes.bfloat16
    woff = np.ascontiguousarray(
        offset_w.reshape(18, 64, 9).transpose(1, 2, 0)
    ).astype(bf16)
    obc = offset_b.reshape(18, 1).astype(np.float32)
    sela = np.zeros((18, 81), np.float32)
    selb = np.zeros((18, 81), np.float32)
    nty = np.zeros((81, 1), np.float32)
    ntx = np.zeros((81, 1), np.float32)
    for k in range(9):
        for i, tyv in enumerate((-1, 0, 1)):
            for j, txv in enumerate((-1, 0, 1)):
                t = 9 * k + 3 * i + j
                sela[2 * k, t] = 1.0
                selb[2 * k + 1, t] = 1.0
                nty[t, 0] = -float(tyv)
                ntx[t, 0] = -float(txv)
    sela = sela.astype(bf16)
    selb = selb.astype(bf16)
    wk = np.ascontiguousarray(
        weight.reshape(64, 64, 9).transpose(1, 2, 0)
    ).astype(bf16)
    wop = np.zeros((128, 3, 18), np.float32)
    wop[0:64] = woff.astype(np.float32)[:, 0::3, :]
    wop[64:128] = woff.astype(np.float32)[:, 1::3, :]
    wop = wop.astype(bf16)
    ident = np.eye(128, dtype=np.float32).astype(bf16)
    bcol = np.tile(bias, 2).reshape(128, 1).astype(np.float32)

    in_maps = []
    for core in range(NCORES):
        bb, half = core // 2, core % 2
        r0 = 64 * half
        xe = np.zeros((128, 72, 136), np.float32)
        rlo, rhi = r0 - 4, r0 + 68
        slo, shi = max(rlo, 0), min(rhi, H)
        xe[0:64, slo - rlo : shi - rlo, 4 : 4 + W] = x[bb, :, slo:shi, :]
        xe[64:128, :, 0:135] = xe[0:64, :, 1:136]
        in_maps.append(dict(
            xe=xe.astype(bf16), woff=woff, wop=wop, obc=obc, sela=sela,
            selb=selb, nty=nty, ntx=ntx, wk=wk, ident=ident, bcol=bcol,
        ))
    return in_maps


def kernel(x, offset_w, offset_b, weight, bias):
    x = np.asarray(x, np.float32)
    offset_w = np.asarray(offset_w, np.float32)
    offset_b = np.asarray(offset_b, np.float32)
    weight = np.asarray(weight, np.float32)
    bias = np.asarray(bias, np.float32)

    from concourse.bass_utils import run_bass_kernel_spmd

    nc = build_program()
    in_maps = _host_inputs(x, offset_w, offset_b, weight, bias)
    res = run_bass_kernel_spmd(nc, in_maps, core_ids=list(range(NCORES)))
    _cached["exec_time_ns"] = res.exec_time_ns

    out = np.zeros((B, O, H, W), np.float32)
    for core in range(NCORES):
        raw = res.results[core]["out"]
        bb, half = core // 2, core % 2
        r0 = 64 * half
        v = raw.reshape(2, 64, 4, 8, 128)
        v = v.transpose(1, 0, 2, 3, 4).reshape(64, 64, 128)
        out[bb, :, r0 : r0 + 64, :] = v
    return out


if __name__ == "__main__":
    xs = {
        "x": np.random.randn(B, C, H, W).astype(np.float32),
        "offset_w": (np.random.randn(18, 64, 3, 3) * 0.01).astype(np.float32),
        "offset_b": (np.random.randn(18) * 0.01).astype(np.float32),
        "weight": (np.random.randn(64, 64, 3, 3) / np.sqrt(576)).astype(np.float32),
        "bias": (np.random.randn(64) * 0.01).astype(np.float32),
    }
    r = kernel(**xs)
    print(r.shape, np.abs(r).max())


# revision 4
# speedup vs baseline: 1.0320x; 1.0073x over previous
"""DeformConv2d Trainium2 Bass kernel, v2.

vs baseline:
  - cj/uy/ux quartered per (mp, mi) so replication DMAs start ~12us in
    (tile-granular deps no longer serialize on the whole cj tensor).
  - 9 hat-product rows per (k, mi) split across engines:
      ty0/ty1 triples -> DVE products on bf16 cj (fast 2x mode)
      ty1/ty2 triples on k%3==0 -> DVE products on fp8 cj (DMA bytes halved)
      ty2 triple otherwise       -> Pool products on fp8 cj
  - products fused per ty-triple: one [128, 3, 8, 128] tensor_mul.
  - cjr replication DMAs issue from SP/Act HWDGE queues (no Pool descgen).
  - phase-1 for quarters q2/q3 interleaved into mp0's modulation stream.

Sharding: 8 cores = (batch b in 0..3) x (row-half in 0..1), as baseline.
"""

import sys
import numpy as np
import ml_dtypes

sys.path.insert(0, "/opt/trn_rl_repo")

B, C, H, W = 4, 64, 128, 128
O = 64
NCORES = 8

_cached = {}


def _assign(k, mi, ty):
    """engine/dtype tag for the ty-triple of slot (k, mi): 'db' bf16-DVE,
    'd8' fp8-DVE, 'p8' fp8-Pool. Pattern A/B/A/C keeps adjacent slots'
    bottleneck engines different so shallow rings can pipeline them."""
    v = (k + mi) % 4
    if k == 8:
        v = 3  # no Pool work in the last tap: fast drain into the mp barrier
    if ty == 0:
        return "db"
    if ty == 1:
        return "d8" if v == 1 else "db"
    return "d8" if v == 3 else "p8"


def build_program():
    if "nc" in _cached:
        return _cached["nc"]
    import concourse.bass as bass
    import concourse.tile as tile
    import concourse.mybir as mybir
    from contextlib import ExitStack
    import bass_rust as _br

    dt = mybir.dt
    AF = mybir.ActivationFunctionType
    ALU = mybir.AluOpType

    nc = bass.Bass()

    xe_d = nc.dram_tensor("xe", [128, 72, 136], dt.bfloat16, kind="ExternalInput")
    wop_d = nc.dram_tensor("wop", [128, 3, 18], dt.bfloat16, kind="ExternalInput")
    woff_d = nc.dram_tensor("woff", [64, 9, 18], dt.bfloat16, kind="ExternalInput")
    ob_d = nc.dram_tensor("obc", [18, 1], dt.float32, kind="ExternalInput")
    sela_d = nc.dram_tensor("sela", [18, 81], dt.bfloat16, kind="ExternalInput")
    selb_d = nc.dram_tensor("selb", [18, 81], dt.bfloat16, kind="ExternalInput")
    nty_d = nc.dram_tensor("nty", [81, 1], dt.float32, kind="ExternalInput")
    ntx_d = nc.dram_tensor("ntx", [81, 1], dt.float32, kind="ExternalInput")
    wk_d = nc.dram_tensor("wk", [64, 9, 64], dt.bfloat16, kind="ExternalInput")
    ident_d = nc.dram_tensor("ident", [128, 128], dt.bfloat16, kind="ExternalInput")
    bcol_d = nc.dram_tensor("bcol", [128, 1], dt.float32, kind="ExternalInput")
    out_d = nc.dram_tensor("out", [128, 4096], dt.float32, kind="ExternalOutput")

    with tile.TileContext(nc) as tc, ExitStack() as ctx:
        const_pool = ctx.enter_context(tc.tile_pool(name="consts", bufs=1))
        xe = const_pool.tile([128, 72, 136], dt.bfloat16)
        nc.sync.dma_start(xe[:], xe_d[:])
        wop = const_pool.tile([128, 3, 18], dt.bfloat16)
        nc.sync.dma_start(wop[:], wop_d[:])
        woff = const_pool.tile([64, 9, 18], dt.bfloat16)
        nc.sync.dma_start(woff[:], woff_d[:])
        obc = const_pool.tile([18, 1], dt.float32)
        nc.sync.dma_start(obc[:], ob_d[:])
        sela = const_pool.tile([18, 81], dt.bfloat16)
        nc.sync.dma_start(sela[:], sela_d[:])
        selb = const_pool.tile([18, 81], dt.bfloat16)
        nc.sync.dma_start(selb[:], selb_d[:])
        nty = const_pool.tile([81, 1], dt.float32)
        nc.sync.dma_start(nty[:], nty_d[:])
        ntx = const_pool.tile([81, 1], dt.float32)
        nc.sync.dma_start(ntx[:], ntx_d[:])
        wk = const_pool.tile([64, 9, 64], dt.bfloat16)
        nc.sync.dma_start(wk[:], wk_d[:])
        ident = const_pool.tile([128, 128], dt.bfloat16)
        nc.sync.dma_start(ident[:], ident_d[:])
        bcol = const_pool.tile([128, 1], dt.float32)
        nc.sync.dma_start(bcol[:], bcol_d[:])

        # per-quarter cj tensors. cjq [81, 2048] = [ph0 1024 | ph1 1024] is
        # the hat-product staging layout; a SWDGE reshape turns it into
        # cjT2 [54 = (k,ty)x2+ph, 3072 = (tx,f)] whose free-contiguous
        # triples make the o-replication DMAs legal 3-dim HWDGE copies.
        cjq_pool = ctx.enter_context(tc.tile_pool(name="cjq", bufs=2))
        cj_pool = ctx.enter_context(tc.tile_pool(name="cj", bufs=1))
        cjT = [cj_pool.tile([54, 3, 1024], dt.bfloat16, name=f"cjT{g}")
               for g in range(4)]
        cjT8 = [cj_pool.tile([54, 3, 1024], dt.float8e4, name=f"cjT8{g}")
                for g in range(4)]

        op_pool = ctx.enter_context(tc.tile_pool(name="p1psum", bufs=2,
                                                 space="PSUM"))
        sb1_pool = ctx.enter_context(tc.tile_pool(name="p1sb", bufs=2))

        def phase1_quarter(g):
            # chunks of this quarter: ph0: 2g, 2g+1 ; ph1: 8+2g, 8+2g+1
            offs_sb = sb1_pool.tile([18, 4, 512], dt.bfloat16, tag="offs",
                                    name=f"offs{g}")
            uyq = sb1_pool.tile([81, 2048], dt.bfloat16, tag="uy",
                                name=f"uy{g}")
            uxq = sb1_pool.tile([81, 2048], dt.bfloat16, tag="ux",
                                name=f"ux{g}")
            chs = [2 * g, 2 * g + 1, 8 + 2 * g, 8 + 2 * g + 1]
            for ci, ch in enumerate(chs):
                po = op_pool.tile([81, 512], dt.float32, tag="p1", name="po")
                for ky in range(3):
                    ay = ky - 1
                    rhs = xe[:, 4 + 4 * ch + ay : 4 + 4 * ch + ay + 4,
                             3 : 3 + 128]
                    nc.tensor.matmul(po[0:18, :], wop[:, ky, :], rhs,
                                     start=(ky == 0), stop=False)
                for ky in range(3):
                    ay = ky - 1
                    rhs = xe[0:64, 4 + 4 * ch + ay : 4 + 4 * ch + ay + 4,
                             5 : 5 + 128]
                    nc.tensor.matmul(po[0:18, :], woff[:, 3 * ky + 2, :], rhs,
                                     start=False, stop=(ky == 2))
                nc.scalar.activation(offs_sb[:, ci, :], po[0:18, :],
                                     AF.Identity, bias=obc[:], scale=1.0)
                pa = op_pool.tile([81, 512], dt.float32, tag="p1", name="pa")
                nc.tensor.matmul(pa[:], sela[:], offs_sb[:, ci, :],
                                 start=True, stop=True)
                nc.scalar.activation(uyq[:, 512 * ci : 512 * (ci + 1)], pa[:],
                                     AF.Abs, bias=nty[:], scale=1.0)
                pb = op_pool.tile([81, 512], dt.float32, tag="p1", name="pb")
                nc.tensor.matmul(pb[:], selb[:], offs_sb[:, ci, :],
                                 start=True, stop=True)
                nc.scalar.activation(uxq[:, 512 * ci : 512 * (ci + 1)], pb[:],
                                     AF.Abs, bias=ntx[:], scale=1.0)
            nc.vector.tensor_scalar(uyq[:], uyq[:], 1.0, 0.0,
                                    ALU.subtract, ALU.min)
            nc.vector.tensor_scalar(uxq[:], uxq[:], 1.0, 0.0,
                                    ALU.subtract, ALU.min)
            cjq = cjq_pool.tile([81, 2048], dt.bfloat16, tag="cjq",
                                name=f"cjq{g}")
            nc.vector.tensor_mul(cjq[:], uyq[:], uxq[:])
            # reshape [t=9k+3ty+tx, (ph,f)] -> [27*ph + 3k+ty, tx, f]:
            # one DMA per (ph, tx): dim0-only partition strides (HWDGE ok)
            for ph in range(2):
                for j in range(3):
                    src_ap = cjq[j : min(j + 79, 81),
                                 1024 * ph : 1024 * (ph + 1)].copy()
                    sp = src_ap.ap[0][0]
                    src_ap.ap = _br.VecI64Pair([[3 * sp, 27], [1, 1024]])
                    nc.sync.dma_start(
                        cjT[g][27 * ph : 27 * ph + 27, j, :], src_ap)
            nc.scalar.activation(cjT8[g][:], cjT[g][:], AF.Copy, scale=1.0)

        zp_pool = ctx.enter_context(tc.tile_pool(name="zpsum", bufs=2,
                                                 space="PSUM"))
        acc_pool = ctx.enter_context(tc.tile_pool(name="acc", bufs=1,
                                                  space="PSUM"))
        zblk_pool = ctx.enter_context(tc.tile_pool(name="zblk", bufs=3))
        cjr_pool = ctx.enter_context(tc.tile_pool(name="cjr", bufs=4))
        cjr8p_pool = ctx.enter_context(tc.tile_pool(name="cjr8p", bufs=6))
        prod_pool = ctx.enter_context(tc.tile_pool(name="prod", bufs=3))
        prods_pool = ctx.enter_context(tc.tile_pool(name="prods", bufs=6))
        outsb_pool = ctx.enter_context(tc.tile_pool(name="outsb", bufs=1))

        def emit_zblk(mp, k):
            zblk = zblk_pool.tile([128, 20, 132], dt.bfloat16, name="zblk")
            for rc in range(5):
                zp = zp_pool.tile([128, 512], dt.float32, tag="zp", name="zp")
                for ph in range(2):
                    xrow = ph * 32 + 16 * mp + 4 * rc + 2
                    rhs = xe[0:64, xrow : xrow + 4, 2 : 2 + 128]
                    nc.tensor.matmul(zp[64 * ph : 64 * ph + 64, :], wk[:, k, :],
                                     rhs, start=True, stop=True)
                nc.scalar.activation(
                    zblk[:, 4 * rc : 4 * rc + 4, 0:128],
                    zp[:].rearrange("p (a b) -> p a b", a=4), AF.Copy,
                    scale=1.0)
            zps = zp_pool.tile([128, 20, 4], dt.float32, tag="zp", name="zps")
            for ph in range(2):
                xrow = ph * 32 + 16 * mp + 2
                rhs = xe[0:64, xrow : xrow + 20, 130 : 134]
                nc.tensor.matmul(zps[64 * ph : 64 * ph + 64, :, :], wk[:, k, :],
                                 rhs, start=True, stop=True)
            nc.scalar.activation(zblk[:, :, 128:132], zps[:], AF.Copy,
                                 scale=1.0)
            return zblk

        def rep_dma(queue_dma, dst, srcT, k, ty):
            """replicate the (k, ty) triple (rows 3k+ty and 27+3k+ty of
            srcT, free [tx, f] = 3072) into dst [128, 3, 1024]: partition
            p = ph*64+o reads srcT row 27*ph + 3k+ty. The 28-row slice
            over-claims the region (safe: conservative hazards only)."""
            r0 = 3 * k + ty
            src_ap = srcT[r0 : r0 + 28, :, :].copy()
            pitch = src_ap.ap[0][0]
            src_ap.ap = _br.VecI64Pair([[27 * pitch, 2], [0, 64], [1, 3072]])
            queue_dma(dst[:], src_ap)

        def emit_prods(mp, k, mi, zblk, srcs):
            """srcs: {ty: (tile, row_ofs, engine_tag)}. Emits the DVE/Pool
            products; returns the per-triple prod list for emit_accums."""
            ky, kx = k // 3, k % 3
            out = []
            # Pool triples first: the slow Pool engine gets a slot of lead
            # time; DVE fills in behind it
            order = sorted(range(3), key=lambda t: srcs[t][2] != "p8")
            for ty in order:
                src, ofs, tag = srcs[ty]
                rb = 8 * mi + ky + ty
                pool_op = tag == "p8"
                # Pool runs singles (a 6us fused triple would stall the PE
                # accum stream); DVE runs the fused triple.
                tls = [(0, 3)] if not pool_op else [(0, 1), (1, 2), (2, 3)]
                for lo, hi in tls:
                    nt = hi - lo
                    if pool_op:
                        prod = prods_pool.tile([128, 1, 8, 128], dt.bfloat16,
                                               name="prodS")
                    else:
                        prod = prod_pool.tile([128, 3, 8, 128], dt.bfloat16,
                                              name="prod")
                    # slice claims the exact read region (cols kx+lo ..
                    # +127+nt) so hazards cover the zblk edge-column writes
                    z_ap = zblk[:, rb : rb + 8,
                                kx + lo : kx + lo + 127 + nt].copy()
                    zpitch = z_ap.ap[0][0]
                    z_ap.ap = _br.VecI64Pair(
                        [[zpitch, 128], [1, nt], [132, 8], [1, 128]])
                    c_ap = src[:, ofs + lo : ofs + hi, :]
                    eng = nc.gpsimd if pool_op else nc.vector
                    eng.tensor_mul(prod[:, 0:nt, :, :], z_ap, c_ap)
                    out.append((prod, ty, lo, nt))
            return out

        def emit_accums(k, prods, acc):
            nmm = sum(nt for _, _, _, nt in prods)
            i = 0
            for prod, ty, lo, nt in prods:
                for tl in range(nt):
                    i += 1
                    pf = prod[:, tl, :, :].rearrange("p a b -> p (a b)")
                    for nchunk in range(2):
                        nc.tensor.matmul(
                            acc[:, 512 * nchunk : 512 * (nchunk + 1)],
                            ident[:],
                            pf[:, 512 * nchunk : 512 * (nchunk + 1)],
                            start=(k == 0 and i == 1),
                            stop=(k == 8 and i == nmm),
                            skip_group_check=True)

        # ---------------- schedule ----------------------------------------
        phase1_quarter(0)
        phase1_quarter(1)

        for mp in range(2):
            acc0 = acc_pool.tile([128, 1024], dt.float32, tag="acc0")
            acc1 = acc_pool.tile([128, 1024], dt.float32, tag="acc1")
            accs = [acc0, acc1]
            zblks = {0: emit_zblk(mp, 0), 1: emit_zblk(mp, 1)}
            pending = None  # (k, prods, acc) accums lag one slot behind
            for k in range(9):
                # hoist zblk two taps ahead: its PE matmuls land before
                # accums(k) in the PE queue, its Act evacs overlap them, so
                # the k+2 products never wait on zblk latency
                if k + 2 <= 8:
                    zblks[k + 2] = emit_zblk(mp, k + 2)
                if mp == 0 and k == 2:
                    phase1_quarter(2)
                if mp == 0 and k == 5:
                    phase1_quarter(3)
                zblk = zblks.pop(k)
                for mi in range(2):
                    g = 2 * mp + mi
                    srcs = {}
                    tyo = sorted(range(3),
                                 key=lambda t: _assign(k, mi, t) != "p8")
                    for ty in tyo:
                        tag = _assign(k, mi, ty)
                        if tag == "db":
                            cjr = cjr_pool.tile([128, 3, 1024], dt.bfloat16,
                                                tag="cjrB", name="cjrB")
                            rep_dma(nc.sync.dma_start, cjr, cjT[g], k, ty)
                        elif tag == "d8":
                            cjr = cjr_pool.tile([128, 3, 1024], dt.float8e4,
                                                tag="cjr8d", name="cjr8d")
                            rep_dma(nc.scalar.dma_start, cjr, cjT8[g], k, ty)
                        else:
                            cjr = cjr8p_pool.tile([128, 3, 1024], dt.float8e4,
                                                  tag="cjr8p", name="cjr8p")
                            rep_dma(nc.scalar.dma_start, cjr, cjT8[g], k, ty)
                        srcs[ty] = (cjr, 0, tag)
                    prods = emit_prods(mp, k, mi, zblk, srcs)
                    if pending is not None:
                        emit_accums(*pending)
                    pending = (k, prods, accs[mi])
            emit_accums(*pending)

            for mi in range(2):
                m = 2 * mp + mi
                osb = outsb_pool.tile([128, 1024], dt.float32)
                nc.scalar.activation(osb[:], accs[mi][:], AF.Identity,
                                     bias=bcol[:], scale=1.0)
                nc.sync.dma_start(out_d[:, 1024 * m : 1024 * (m + 1)], osb[:])

    _patch_multiwait(nc)
    _cached["nc"] = nc
    return nc


def _patch_multiwait(nc):
    """walrus here accepts one sync-wait per instruction; split extras onto
    injected same-engine Drain carriers (waiting earlier is always safe)."""
    import json
    import types

    orig = nc.to_json_bytes

    def patched(self):
        bir = json.loads(orig())
        uid = [0]
        for fn in bir["functions"]:
            for blk in fn["blocks"]:
                out = []
                for ins in blk["instructions"]:
                    si = ins.get("sync_info")
                    ow = (si or {}).get("on_wait") or []
                    if len(ow) > 1:
                        for w in ow[:-1]:
                            uid[0] += 1
                            out.append({
                                "debug": ins.get("debug", 0),
                                "engine": ins["engine"],
                                "ins": [], "outs": [],
                                "name": f"WSPL-{uid[0]}",
                                "opcode": "Drain",
                                "sync_info": {"on_update": [],
                                              "on_wait": [w]},
                            })
                        si["on_wait"] = [ow[-1]]
                    out.append(ins)
                blk["instructions"] = out
        return json.dumps(bir).encode()

    nc.to_json_bytes = types.MethodType(patched, nc)


def _host_inputs(x, offset_w, offset_b, weight, bias):
    bf16 = ml_dtypes.bfloat16
    woff = np.ascontiguousarray(
        offset_w.reshape(18, 64, 9).transpose(1, 2, 0)
    ).astype(bf16)
    obc = offset_b.reshape(18, 1).astype(np.float32)
    sela = np.zeros((18, 81), np.float32)
    selb = np.zeros((18, 81), np.float32)
    nty = np.zeros((81, 1), np.float32)
    ntx = np.zeros((81, 1), np.float32)
    for k in range(9):
        for i, tyv in enumerate((-1, 0, 1)):
            for j, txv in enumerate((-1, 0, 1)):
                t = 9 * k + 3 * i + j
                sela[2 * k, t] = 1.0
                selb[2 * k + 1, t] = 1.0
                nty[t, 0] = -float(tyv)
                ntx[t, 0] = -float(txv)
    sela = sela.astype(bf16)
    selb = selb.astype(bf16)
    wk = np.ascontiguousarray(
        weight.reshape(64, 64, 9).transpose(1, 2, 0)
    ).astype(bf16)
    wop = np.zeros((128, 3, 18), np.float32)
    wop[0:64] = woff.astype(np.float32)[:, 0::3, :]
    wop[64:128] = woff.astype(np.float32)[:, 1::3, :]
    wop = wop.astype(bf16)
    ident = np.eye(128, dtype=np.float32).astype(bf16)
    bcol = np.tile(bias, 2).reshape(128, 1).astype(np.float32)

    in_maps = []
    for core in range(NCORES):
        bb, half = core // 2, core % 2
        r0 = 64 * half
        xe = np.zeros((128, 72, 136), np.float32)
        rlo, rhi = r0 - 4, r0 + 68
        slo, shi = max(rlo, 0), min(rhi, H)
        xe[0:64, slo - rlo : shi - rlo, 4 : 4 + W] = x[bb, :, slo:shi, :]
        xe[64:128, :, 0:135] = xe[0:64, :, 1:136]
        in_maps.append(dict(
            xe=xe.astype(bf16), woff=woff, wop=wop, obc=obc, sela=sela,
            selb=selb, nty=nty, ntx=ntx, wk=wk, ident=ident, bcol=bcol,
        ))
    return in_maps


def kernel(x, offset_w, offset_b, weight, bias):
    x = np.asarray(x, np.float32)
    offset_w = np.asarray(offset_w, np.float32)
    offset_b = np.asarray(offset_b, np.float32)
    weight = np.asarray(weight, np.float32)
    bias = np.asarray(bias, np.float32)

    from concourse.bass_utils import run_bass_kernel_spmd

    nc = build_program()
    in_maps = _host_inputs(x, offset_w, offset_b, weight, bias)
    res = run_bass_kernel_spmd(nc, in_maps, core_ids=list(range(NCORES)))
    _cached["exec_time_ns"] = res.exec_time_ns

    out = np.zeros((B, O, H, W), np.float32)
    for core in range(NCORES):
        raw = res.results[core]["out"]
        bb, half = core // 2, core % 2
        r0 = 64 * half
        v = raw.reshape(2, 64, 4, 8, 128)
        v = v.transpose(1, 0, 2, 3, 4).reshape(64, 64, 128)
        out[bb, :, r0 : r0 + 64, :] = v
    return out


if __name__ == "__main__":
    xs = {
        "x": np.random.randn(B, C, H, W).astype(np.float32),
        "offset_w": (np.random.randn(18, 64, 3, 3) * 0.01).astype(np.float32),
        "offset_b": (np.random.randn(18) * 0.01).astype(np.float32),
        "weight": (np.random.randn(64, 64, 3, 3) / np.sqrt(576)).astype(np.float32),
        "bias": (np.random.randn(64) * 0.01).astype(np.float32),
    }
    r = kernel(**xs)
    print(r.shape, np.abs(r).max())
